# revision 24
# baseline (speedup 1.0000x reference)
"""Joseph 3D projector on 8 TRN2 NeuronCores — merged banded matmuls.

Formulation: for each angle a, out[u, v] = sum_p (DT*M_a)[p, u] * volT[p, v]
where p = y*128 + x and volT[p, v] = vol[0,0,x,y,v].  M_a is banded: per
contraction slab (y-line or x-line) the nonzero u's lie in a narrow window.

Measured TRN2 matmul cost is max(~33ns [per-MM LDWEIGHTS port], 5ns +
N/2.42 [column stream]), so many thin matmuls are LDWEIGHTS-floor-bound.
v4 therefore MERGES three adjacent-angle slots into ONE matmul per slab:
slots are given disjoint psum territories (psum col = G_k + u) inside a
slab-range (8 slabs) so windows never alias, and each (group, range) psum
region is drained (added into an f32 accumulator) while the next range
computes in the other bank of a ping/pong pair.  Dead columns between
windows stream zeros, which are harmless under psum first-touch semantics.

Mirror sharing (exact): s(k) = (60-k) mod 120 maps M_k to M_{s(k)} by an
x<->y transpose (plain, k in [61,119]) or transpose + u-flip (k in [0,60]).
The merged B-group block is shared with its flip group via a reversed rhs
AP; the merged A-group block is shared plainly.  Stored M ~9.5 MB/core.

Sharding: 15 angle-slots; slot j on core c holds angle SLOTS[j][0][c]; all
8 cores share one SPMD program whose window geometry covers each slot's 8
angles.
"""
import numpy as np
import ml_dtypes

D = H = W = 128
V = U = 128
A = 120
S = 128
NCORES = 8
NSLOT = A // NCORES  # 15
T = 0.5 * float(np.sqrt(((W - 1) * 1.0) ** 2 + ((H - 1) * 1.0) ** 2))
DT = 2.0 * T / S

RSZ = 8            # slab-range size for merged groups
NR = S // RSZ      # 16 ranges


# ---- slot table ------------------------------------------------------------
def _slot_table():
    slots = []
    # bases: B0,B1,B2 (axis0), A0,A1,A2 (axis1)
    for b, a0 in enumerate((0, 8, 16)):
        slots.append((tuple(a0 + c for c in range(8)), b, None))
    for b, a0 in zip((3, 4, 5), (61, 69, 77)):
        slots.append((tuple(a0 + c for c in range(8)), b, None))
    slots.append((tuple(24 + c for c in range(8)), 6, None))     # 6: U0 axis0
    slots.append((tuple(85 + c for c in range(8)), 7, None))     # 7: U1 axis1
    slots.append(((32, 33, 34, 35, 36, 93, 94, 95), 8, None))    # 8: U2 axis1
    # mirrors 9..14: B0m,B1m,B2m (flip), A0m,A1m,A2m (plain)
    for j, kind in ((0, "flip"), (1, "flip"), (2, "flip"),
                    (3, "plain"), (4, "plain"), (5, "plain")):
        base_angs = slots[j][0]
        m_angs = tuple((60 - k) % 120 for k in base_angs)
        slots.append((m_angs, slots[j][1], kind))
    return slots

SLOTS = _slot_table()

# merged groups: (member slot ids in G-packing order, stored?, share kind)
#   MG0 (ph0, stored)   = [B0, B1, B2]
#   MG1 (ph1, flip      = [B2m, B1m, B0m] reads MG0's block reversed
#   MG2 (ph1, stored)   = [A0, A1, A2]
#   MG3 (ph0, plain)    = [A2m, A1m, A0m] reads MG2's block as-is
MGROUPS = [
    dict(slots=[0, 1, 2], phase=0, store=True,  src=0, kind=None, rsz=16),
    dict(slots=[11, 10, 9], phase=1, store=False, src=0, kind="flip", rsz=16),
    dict(slots=[3, 4, 5], phase=1, store=True,  src=2, kind=None, rsz=16),
    dict(slots=[12, 13, 14], phase=0, store=False, src=2, kind="plain", rsz=16),
    dict(slots=[7, 8], phase=1, store=True, src=4, kind=None, rsz=16),
]
SINGLES = [dict(slot=6, phase=0)]


def _build_M(cos_t, sin_t):
    u_phys = np.arange(U, dtype=np.float64) - (U - 1) / 2.0
    t = -T + (np.arange(S, dtype=np.float64) + 0.5) * DT
    x_idx = (-u_phys[None, :] * sin_t + t[:, None] * cos_t) + (W - 1) / 2.0
    y_idx = (u_phys[None, :] * cos_t + t[:, None] * sin_t) + (H - 1) / 2.0
    x0 = np.floor(x_idx).astype(np.int64)
    y0 = np.floor(y_idx).astype(np.int64)
    wx = x_idx - x0
    wy = y_idx - y0
    Mflat = np.zeros(H * W * U, np.float32)
    uu = np.broadcast_to(np.arange(U, dtype=np.int64)[None, :], (S, U))
    for dy, dx in ((0, 0), (0, 1), (1, 0), (1, 1)):
        yi = y0 + dy
        xi = x0 + dx
        w = (wy if dy else 1 - wy) * (wx if dx else 1 - wx)
        valid = (xi >= 0) & (xi <= W - 1) & (yi >= 0) & (yi <= H - 1)
        p = np.clip(yi, 0, H - 1) * W + np.clip(xi, 0, W - 1)
        flat = (p * U + uu)[valid]
        Mflat += np.bincount(flat, weights=w[valid].astype(np.float64),
                             minlength=H * W * U).astype(np.float32)
    return Mflat.reshape(H * W, U)


def _slot_axis(angs):
    th = np.array(angs, np.float64) * (np.pi / A)
    return 0 if np.mean(np.abs(np.sin(th))) <= np.mean(np.abs(np.cos(th))) else 1


def _union_geom(Ms, angs, ax):
    lo = np.full(S, U, np.int64)
    hi = np.full(S, -1, np.int64)
    for k in angs:
        Mr = Ms[k].reshape(H, W, U)
        sl = Mr if ax == 0 else Mr.transpose(1, 0, 2)
        nz = sl.any(axis=1)
        any_s = nz.any(axis=1)
        first = nz.argmax(axis=1)
        last = U - 1 - nz[:, ::-1].argmax(axis=1)
        lo = np.where(any_s, np.minimum(lo, first), lo)
        hi = np.where(any_s, np.maximum(hi, last), hi)
    wv = np.maximum(hi - lo + 1, 0).astype(np.int64)
    lo = np.where(hi < 0, 0, lo).astype(np.int64)
    return wv, lo


def _schedule(angles):
    Ms = np.stack([_build_M(np.cos(np.float64(a)), np.sin(np.float64(a)))
                   for a in angles])
    axes, widths, offs = [], [], []
    for j, (angs, blk, kind) in enumerate(SLOTS):
        ax = _slot_axis(angs)
        wv, lo = _union_geom(Ms, angs, ax)
        if kind is not None:
            base_wv, base_lo = widths[blk], offs[blk]
            assert (wv == base_wv).all(), f"slot {j}: mirror width mismatch"
            exp_lo = (np.where(base_wv > 0, U - base_lo - base_wv, lo)
                      if kind == "flip" else base_lo)
            assert (np.where(wv > 0, lo == exp_lo, True)).all(), \
                f"slot {j}: mirror offs mismatch"
            lo = exp_lo
        cover = np.zeros(U, bool)
        for s in range(S):
            if wv[s] > 0:
                cover[lo[s]:lo[s] + wv[s]] = True
        assert cover.all(), f"slot {j}: uncovered out cols"
        axes.append(ax)
        widths.append(wv)
        offs.append(lo)
    return Ms, np.array(axes), np.stack(widths), np.stack(offs)


def _group_geom(widths, offs, g):
    """Per-range G packing for a merged group.

    Returns per range r: G[k] per member, Tspan, and per slab s in range:
    (mn, mx) merged span in region coords (mn>mx if empty); plus per-member
    drain runs [(g_lo, g_hi, u_lo)] in region coords.
    """
    js = g["slots"]
    rsz = g["rsz"]
    ranges = []
    for r in range(S // rsz):
        r0, r1 = r * rsz, (r + 1) * rsz
        G = []
        cur = 0
        terr = []
        for j in js:
            ss = [s for s in range(r0, r1) if widths[j][s] > 0]
            if not ss:
                G.append(None)
                terr.append(None)
                continue
            tlo = min(int(offs[j][s]) for s in ss)
            thi = max(int(offs[j][s] + widths[j][s] - 1) for s in ss)
            G.append(cur - tlo)
            terr.append((cur, cur + thi - tlo))
            cur = cur + (thi - tlo) + 1
        Tspan = cur
        spans = []
        for s in range(r0, r1):
            mn, mx = None, None
            for k, j in enumerate(js):
                if widths[j][s] > 0:
                    a = G[k] + int(offs[j][s])
                    b = a + int(widths[j][s]) - 1
                    mn = a if mn is None else min(mn, a)
                    mx = b if mx is None else max(mx, b)
            spans.append((mn, mx) if mn is not None else None)
        # drain runs per member: covered cols within territory
        runs = []
        for k, j in enumerate(js):
            if G[k] is None:
                runs.append([])
                continue
            covered = np.zeros(Tspan, bool)
            for s in range(r0, r1):
                if widths[j][s] > 0:
                    a = G[k] + int(offs[j][s])
                    covered[a:a + int(widths[j][s])] = True
            rr = []
            i = 0
            while i < Tspan:
                if covered[i]:
                    i2 = i
                    while i2 + 1 < Tspan and covered[i2 + 1]:
                        i2 += 1
                    rr.append((i, i2, i - G[k]))
                    i = i2 + 1
                else:
                    i += 1
            runs.append(rr)
        ranges.append(dict(G=G, Tspan=Tspan, spans=spans, runs=runs, r0=r0))
    return ranges


_COMPILED = {}


def _get_compiled(angles):
    key = hash(angles.tobytes())
    if key in _COMPILED:
        return _COMPILED[key]
    from contextlib import ExitStack
    import concourse.bacc as bacc
    import concourse.tile as tile
    import concourse.mybir as mybir

    Ms, axes, widths, offs = _schedule(angles)
    for g in MGROUPS:
        ph = g["phase"]
        for j in g["slots"]:
            assert axes[j] == ph, (j, axes[j], ph)
    gg = [_group_geom(widths, offs, g) for g in MGROUPS]
    # mirror consistency: shared groups must be exact reflections (flip) or
    # identical (plain) so one stored block serves both.
    for gi, g in enumerate(MGROUPS):
        if g["store"]:
            continue
        bb = gg[g["src"]]
        mm = gg[gi]
        assert len(bb) == len(mm)
        for r in range(len(bb)):
            assert mm[r]["Tspan"] == bb[r]["Tspan"], (gi, r)
            for si in range(len(bb[r]["spans"])):
                sb_, sm_ = bb[r]["spans"][si], mm[r]["spans"][si]
                assert (sb_ is None) == (sm_ is None), (gi, r, si)
                if sb_ is None:
                    continue
                if g["kind"] == "flip":
                    Tsp = bb[r]["Tspan"]
                    assert sm_[0] == Tsp - 1 - sb_[1], (gi, r, si)
                    assert sm_[1] == Tsp - 1 - sb_[0], (gi, r, si)
                else:
                    assert sm_ == sb_, (gi, r, si)

    # out column order: phase0 merged/singles then phase1
    order = ([j for g in MGROUPS if g["phase"] == 0 for j in g["slots"]]
             + [sg["slot"] for sg in SINGLES if sg["phase"] == 0]
             + [j for g in MGROUPS if g["phase"] == 1 for j in g["slots"]]
             + [sg["slot"] for sg in SINGLES if sg["phase"] == 1])
    out_base = {j: 128 * p for p, j in enumerate(order)}
    ph0_cols = 128 * 7  # phase-0 slots: MG0(3) + MG3(3) + U0
    # merged slots get an f32 accumulator region
    acc_slots = [j for g in MGROUPS for j in g["slots"]]
    acc_base = {j: 128 * p for p, j in enumerate(acc_slots)}

    # stored column layout: m0 per slab = [MG0 span][MG2 span][U0 w];
    # m1 per slab = [U1 w][U2 w]
    def stream_layout(parts):
        # parts: list of per-slab width arrays
        base = np.zeros(S + 1, np.int64)
        cum = []
        for s in range(S):
            c = 0
            offs_ = []
            for warr in parts:
                offs_.append(c)
                c += int(warr[s])
            cum.append(offs_)
            base[s + 1] = base[s] + c
        return base, np.array(cum, np.int64), int(base[S])

    def gspan_width(gi):
        w = np.zeros(S, np.int64)
        for rr in gg[gi]:
            for si, sp in enumerate(rr["spans"]):
                if sp is not None:
                    w[rr["r0"] + si] = sp[1] - sp[0] + 1
        return w

    w_mg0 = gspan_width(0)
    w_mg2 = gspan_width(2)
    w_mg4 = gspan_width(4)
    base0, cum0, W0tot = stream_layout([w_mg0, w_mg2, widths[6]])
    base1, cum1, W1tot = stream_layout([w_mg4])

    nc = bacc.Bacc("TRN2", target_bir_lowering=False, debug=False,
                   enable_asserts=False, num_devices=NCORES)
    bf16 = mybir.dt.bfloat16
    f8 = mybir.dt.float8e3
    f32 = mybir.dt.float32

    volA_d = nc.dram_tensor("volA", [S, H * D], bf16, kind="ExternalInput").ap()
    volB_d = nc.dram_tensor("volB", [S, H * D], bf16, kind="ExternalInput").ap()
    m0_d = nc.dram_tensor("m0", [S, W0tot], f8, kind="ExternalInput").ap()
    m1_d = nc.dram_tensor("m1", [S, W1tot], f8, kind="ExternalInput").ap()
    out_d = nc.dram_tensor("out", [V, NSLOT * U], bf16, kind="ExternalOutput").ap()

    with tile.TileContext(nc) as tc:
        with ExitStack() as ctx:
            sbuf = ctx.enter_context(tc.tile_pool(name="sbuf", bufs=1))
            psum = ctx.enter_context(tc.tile_pool(name="psum", bufs=1, space="PSUM"))

            volA_sb = sbuf.tile([S, H * D], bf16)
            volB_sb = sbuf.tile([S, H * D], bf16)
            m0_sb = sbuf.tile([S, W0tot], f8)
            m1_sb = sbuf.tile([S, W1tot], f8)
            acc_sb = sbuf.tile([V, len(acc_slots) * U], f32)
            out_sb = sbuf.tile([V, NSLOT * U], bf16)
            ps = [psum.tile([V, 512], f32, name=f"ps{b}") for b in range(8)]

            # psum bank plan:
            #  MG0 (ph0): ps0/ps1   MG3 (ph0): ps2/ps3   U0: ps4[0:128]
            #  MG1 (ph1): ps5/ps6   MG2 (ph1): ps0/ps1   MG4 (ph1): ps2/ps3
            #  (cross-phase reuse is safe: the new epoch's matmuls are
            #  ordered behind the old epoch's full-bank scratch copy)
            mg_banks = {0: (0, 1, 2), 3: (5, 6, 7), 1: (5, 6, 7),
                        2: (0, 1, 2), 4: (3, 4)}
            single_bank = {6: (4, 0)}
            bank_w = {b: 136 for b in range(8)}
            for gi2, bks in mg_banks.items():
                mt = max(rr2["Tspan"] for rr2 in gg[gi2])
                for b in bks:
                    bank_w[b] = min(512, max(bank_w[b], mt + 8))

            # zero the merged-slot accumulator (DVE+ACT halves)
            hn = len(acc_slots) * U // 2
            nc.vector.memset(acc_sb[:, 0:hn], 0.0)
            nc.scalar.memzero(acc_sb[:, hn:len(acc_slots) * U])
            # zero all psum banks once: every matmul runs start=False and
            # accumulates onto zeroed psum; epoch boundaries re-zero via the
            # ACT anchor below, so psum first-touch state never matters.
            for b in range(8):
                if b % 2 == 0:
                    nc.vector.memset(ps[b], 0.0)
                else:
                    nc.scalar.memzero(ps[b])
            WARMUP = 24
            for _ in range(WARMUP):
                nc.tensor.matmul(ps[7][:, 0:128], lhsT=acc_sb[:, 0:128],
                                 rhs=acc_sb[:, 128:256], start=True, stop=True)

            # ---- stream DMAs: few big dma_starts, need-ordered, 2 rings
            def vitem(sb_t, d_t, s0, s1):
                return (sb_t[:, s0 * H:s1 * H], d_t[:, s0 * H:s1 * H])

            def mitem(sb_t, d_t, base, s0, s1):
                c0, c1 = int(base[s0]), int(base[s1])
                return (sb_t[:, c0:c1], d_t[:, c0:c1]) if c1 > c0 else None

            ring0 = [vitem(volA_sb, volA_d, 0, 32),
                     mitem(m0_sb, m0_d, base0, 32, 64),
                     vitem(volA_sb, volA_d, 64, 96),
                     mitem(m0_sb, m0_d, base0, 96, 128),
                     vitem(volB_sb, volB_d, 0, 64),
                     mitem(m1_sb, m1_d, base1, 64, 128)]
            ring1 = [mitem(m0_sb, m0_d, base0, 0, 32),
                     vitem(volA_sb, volA_d, 32, 64),
                     mitem(m0_sb, m0_d, base0, 64, 96),
                     vitem(volA_sb, volA_d, 96, 128),
                     mitem(m1_sb, m1_d, base1, 0, 64),
                     vitem(volB_sb, volB_d, 64, 128)]
            for eng, items in ((nc.sync, ring0), (nc.scalar, ring1)):
                for it in items:
                    if it is not None:
                        eng.dma_start(it[0], it[1])

            # ---- start/stop: every (group, range) opens with start=True
            # (clears its ping/pong bank; safe — the prior epoch was drained,
            # and Tile serializes the PE write behind the drain reads) and
            # closes with stop=True on its last matmul.  Singles likewise per
            # slot (each owns its bank region during its phase).
            for phase in (0, 1):
                vol_sb = volA_sb if phase == 0 else volB_sb
                mgs = [gi for gi in range(len(MGROUPS))
                       if MGROUPS[gi]["phase"] == phase]
                sgs = [sg for sg in SINGLES if sg["phase"] == phase]

                def drain_range(gi, rr, bank):
                    grp = MGROUPS[gi]
                    for k, j in enumerate(grp["slots"]):
                        for (g_lo, g_hi, u_lo) in rr["runs"][k]:
                            n = g_hi - g_lo + 1
                            ab = acc_base[j] + u_lo
                            nc.vector.tensor_add(
                                acc_sb[:, ab:ab + n],
                                acc_sb[:, ab:ab + n],
                                ps[bank][:, g_lo:g_hi + 1])
                    # DVE re-zeroes the bank's active region right after its
                    # own drain reads (same engine -> no cross-engine sem):
                    # a full-region write Tile orders after this epoch's
                    # matmuls AND the drain reads, and before the next
                    # epoch's matmuls (all matmuls run start=False onto
                    # zeroed psum).  With the 3-deep bank rotation this
                    # anchor sits ~2 epochs off the tensor critical path.
                    nc.vector.memset(ps[bank][:, 0:bank_w[bank]], 0.0)

                for s in range(S):
                    lhsT = vol_sb[:, s * D:(s + 1) * D]
                    for gi in mgs:
                        grp = MGROUPS[gi]
                        rsz = grp["rsz"]
                        r, si = s // rsz, s % rsz
                        rr = gg[gi][r]
                        sp = rr["spans"][si]
                        if sp is None:
                            continue
                        mn, mx = sp
                        L = mx - mn + 1
                        bank = mg_banks[gi][r % len(mg_banks[gi])]
                        if grp["src"] == 4:
                            c0 = int(base1[s] + cum1[s][0])
                            src_sb = m1_sb
                        else:
                            part = 0 if grp["src"] == 0 else 1
                            c0 = int(base0[s] + cum0[s][part])
                            src_sb = m0_sb
                        rhs = src_sb[:, c0:c0 + L]
                        if grp["kind"] == "flip":
                            rhs = rhs[:, ::-1]
                        is_last = not any(
                            rr["spans"][si2] is not None
                            for si2 in range(si + 1, rsz))
                        nc.tensor.matmul(
                            ps[bank][:, mn:mx + 1], lhsT=lhsT, rhs=rhs,
                            start=False, stop=is_last,
                            skip_group_check=True)
                    for sg in sgs:
                        j = sg["slot"]
                        w = int(widths[j][s])
                        if w == 0:
                            continue
                        bank, cb = single_bank[j]
                        col = cb + int(offs[j][s])
                        c0 = int(base0[s] + cum0[s][2])
                        is_last = not any(
                            widths[j][s2] > 0 for s2 in range(s + 1, S))
                        nc.tensor.matmul(
                            ps[bank][:, col:col + w], lhsT=lhsT,
                            rhs=m0_sb[:, c0:c0 + w], start=False,
                            stop=is_last, skip_group_check=True)
                    # fire drains for every group whose range ends at slab s
                    for gi in mgs:
                        rsz = MGROUPS[gi]["rsz"]
                        if (s + 1) % rsz == 0:
                            r = s // rsz
                            drain_range(gi, gg[gi][r],
                                        mg_banks[gi][r % len(mg_banks[gi])])
                # ---- end of phase: singles drain + merged convert + flush
                for sg in sgs:
                    j = sg["slot"]
                    bank, cb = single_bank[j]
                    nc.scalar.mul(out_sb[:, out_base[j]:out_base[j] + U],
                                  ps[bank][:, cb:cb + U], 1.0)
                    nc.scalar.memzero(ps[bank][:, 0:bank_w[bank]])
                for qe, gi in enumerate(mgs):
                    js = MGROUPS[gi]["slots"]
                    ob = out_base[js[0]]
                    ab = acc_base[js[0]]
                    n = len(js) * U
                    assert all(out_base[j2] == ob + 128 * i2
                               for i2, j2 in enumerate(js))
                    assert all(acc_base[j2] == ab + 128 * i2
                               for i2, j2 in enumerate(js))
                    if qe % 2 == 0:
                        nc.vector.tensor_scalar_mul(
                            out_sb[:, ob:ob + n], acc_sb[:, ab:ab + n], 1.0)
                    else:
                        nc.scalar.mul(out_sb[:, ob:ob + n],
                                      acc_sb[:, ab:ab + n], 1.0)
                lo_, hi_ = (0, ph0_cols) if phase == 0 else (ph0_cols, NSLOT * U)
                mid = (lo_ + hi_) // 2
                nc.sync.dma_start(out_d[:, lo_:mid], out_sb[:, lo_:mid])
                nc.scalar.dma_start(out_d[:, mid:hi_], out_sb[:, mid:hi_])

    nc.compile()
    meta = dict(nc=nc, Ms=Ms, axes=axes, widths=widths, offs=offs,
                gg=gg, order=order, out_base=out_base,
                lay=(base0, cum0, W0tot, base1, cum1, W1tot),
                w_mg=(w_mg0, w_mg2))
    _COMPILED[key] = meta
    return meta


def _pack_core(meta, core):
    """fp8 stored blocks (pre-scaled by DT) for one core."""
    f8 = ml_dtypes.float8_e3m4
    Ms, axes, widths, offs = (meta["Ms"], meta["axes"], meta["widths"],
                              meta["offs"])
    base0, cum0, W0tot, base1, cum1, W1tot = meta["lay"]
    gg = meta["gg"]
    m0 = np.zeros((S, W0tot), f8)
    m1 = np.zeros((S, W1tot), f8)

    def band(j, s):
        k = SLOTS[j][0][core]
        ax = axes[j]
        Mr = Ms[k].reshape(H, W, U)
        sl = Mr if ax == 0 else Mr.transpose(1, 0, 2)
        o = int(offs[j][s])
        w = int(widths[j][s])
        return (DT * sl[s][:, o:o + w]).astype(f8)

    # merged stored groups: MG0 (m0 part 0), MG2 (m0 part 1), MG4 (m1)
    for gi, (m, base, cum, part) in ((0, (m0, base0, cum0, 0)),
                                     (2, (m0, base0, cum0, 1)),
                                     (4, (m1, base1, cum1, 0))):
        grp = MGROUPS[gi]
        for rr in gg[gi]:
            for si, sp in enumerate(rr["spans"]):
                s = rr["r0"] + si
                if sp is None:
                    continue
                mn = sp[0]
                c0 = int(base[s] + cum[s][part])
                for k, j in enumerate(grp["slots"]):
                    if widths[j][s] > 0:
                        a = rr["G"][k] + int(offs[j][s])
                        m[:, c0 + a - mn:c0 + a - mn + int(widths[j][s])] = \
                            band(j, s)
    # single U0
    for s in range(S):
        w = int(widths[6][s])
        if w > 0:
            c0 = int(base0[s] + cum0[s][2])
            m0[:, c0:c0 + w] = band(6, s)
    return m0, m1


def kernel(vol, angles):
    from concourse.bass_utils import run_bass_kernel_spmd

    vol = np.asarray(vol, dtype=np.float32)
    angles = np.asarray(angles, dtype=np.float32)
    meta = _get_compiled(angles)
    nc = meta["nc"]

    volA = vol[0, 0].reshape(S, H * D).astype(ml_dtypes.bfloat16)
    volB = np.ascontiguousarray(vol[0, 0].transpose(1, 0, 2)).reshape(
        S, H * D).astype(ml_dtypes.bfloat16)
    in_maps = []
    for c in range(NCORES):
        m0, m1 = _pack_core(meta, c)
        in_maps.append({"volA": volA, "volB": volB, "m0": m0, "m1": m1})

    res = run_bass_kernel_spmd(nc, in_maps, core_ids=list(range(NCORES)))
    global _LAST_RES
    _LAST_RES = res
    full = np.empty((1, 1, U, A, V), np.float32)
    for c, r in enumerate(res.results):
        rc = r["out"].astype(np.float32)
        for p, j in enumerate(meta["order"]):
            k = SLOTS[j][0][c]
            full[0, 0, :, k, :] = rc[:, p * U:(p + 1) * U].T
    return full


# revision 25
# speedup vs baseline: 1.0748x; 1.0748x over previous
"""Joseph 3D projector on 8 TRN2 NeuronCores — merged banded matmuls.

Formulation: for each angle a, out[u, v] = sum_p (DT*M_a)[p, u] * volT[p, v]
where p = y*128 + x and volT[p, v] = vol[0,0,x,y,v].  M_a is banded: per
contraction slab (y-line or x-line) the nonzero u's lie in a narrow window.

Measured TRN2 matmul cost is max(~33ns [per-MM LDWEIGHTS port], 5ns +
N/2.42 [column stream]), so many thin matmuls are LDWEIGHTS-floor-bound.
v4 therefore MERGES three adjacent-angle slots into ONE matmul per slab:
slots are given disjoint psum territories (psum col = G_k + u) inside a
slab-range (8 slabs) so windows never alias, and each (group, range) psum
region is drained (added into an f32 accumulator) while the next range
computes in the other bank of a ping/pong pair.  Dead columns between
windows stream zeros, which are harmless under psum first-touch semantics.

Mirror sharing (exact): s(k) = (60-k) mod 120 maps M_k to M_{s(k)} by an
x<->y transpose (plain, k in [61,119]) or transpose + u-flip (k in [0,60]).
The merged B-group block is shared with its flip group via a reversed rhs
AP; the merged A-group block is shared plainly.  Stored M ~9.5 MB/core.

Sharding: 15 angle-slots; slot j on core c holds angle SLOTS[j][0][c]; all
8 cores share one SPMD program whose window geometry covers each slot's 8
angles.
"""
import numpy as np
import ml_dtypes

D = H = W = 128
V = U = 128
A = 120
S = 128
NCORES = 8
NSLOT = A // NCORES  # 15
T = 0.5 * float(np.sqrt(((W - 1) * 1.0) ** 2 + ((H - 1) * 1.0) ** 2))
DT = 2.0 * T / S

RSZ = 8            # slab-range size for merged groups
NR = S // RSZ      # 16 ranges


# ---- slot table ------------------------------------------------------------
def _slot_table():
    slots = []
    # bases: B0,B1,B2 (axis0), A0,A1,A2 (axis1)
    for b, a0 in enumerate((0, 8, 16)):
        slots.append((tuple(a0 + c for c in range(8)), b, None))
    for b, a0 in zip((3, 4, 5), (61, 69, 77)):
        slots.append((tuple(a0 + c for c in range(8)), b, None))
    slots.append((tuple(24 + c for c in range(8)), 6, None))     # 6: U0 axis0
    slots.append((tuple(85 + c for c in range(8)), 7, None))     # 7: U1 axis1
    slots.append(((32, 33, 34, 35, 36, 93, 94, 95), 8, None))    # 8: U2 axis1
    # mirrors 9..14: B0m,B1m,B2m (flip), A0m,A1m,A2m (plain)
    for j, kind in ((0, "flip"), (1, "flip"), (2, "flip"),
                    (3, "plain"), (4, "plain"), (5, "plain")):
        base_angs = slots[j][0]
        m_angs = tuple((60 - k) % 120 for k in base_angs)
        slots.append((m_angs, slots[j][1], kind))
    return slots

SLOTS = _slot_table()

# merged groups: (member slot ids in G-packing order, stored?, share kind)
#   MG0 (ph0, stored)   = [B0, B1, B2]
#   MG1 (ph1, flip      = [B2m, B1m, B0m] reads MG0's block reversed
#   MG2 (ph1, stored)   = [A0, A1, A2]
#   MG3 (ph0, plain)    = [A2m, A1m, A0m] reads MG2's block as-is
MGROUPS = [
    dict(slots=[0, 1, 2], phase=0, store=True,  src=0, kind=None, rsz=16),
    dict(slots=[11, 10, 9], phase=1, store=False, src=0, kind="flip", rsz=16),
    dict(slots=[3, 4, 5], phase=1, store=True,  src=2, kind=None, rsz=16),
    dict(slots=[12, 13, 14], phase=0, store=False, src=2, kind="plain", rsz=16),
    dict(slots=[7, 8], phase=1, store=True, src=4, kind=None, rsz=16),
]
SINGLES = [dict(slot=6, phase=0)]


def _build_M(cos_t, sin_t):
    u_phys = np.arange(U, dtype=np.float64) - (U - 1) / 2.0
    t = -T + (np.arange(S, dtype=np.float64) + 0.5) * DT
    x_idx = (-u_phys[None, :] * sin_t + t[:, None] * cos_t) + (W - 1) / 2.0
    y_idx = (u_phys[None, :] * cos_t + t[:, None] * sin_t) + (H - 1) / 2.0
    x0 = np.floor(x_idx).astype(np.int64)
    y0 = np.floor(y_idx).astype(np.int64)
    wx = x_idx - x0
    wy = y_idx - y0
    Mflat = np.zeros(H * W * U, np.float32)
    uu = np.broadcast_to(np.arange(U, dtype=np.int64)[None, :], (S, U))
    for dy, dx in ((0, 0), (0, 1), (1, 0), (1, 1)):
        yi = y0 + dy
        xi = x0 + dx
        w = (wy if dy else 1 - wy) * (wx if dx else 1 - wx)
        valid = (xi >= 0) & (xi <= W - 1) & (yi >= 0) & (yi <= H - 1)
        p = np.clip(yi, 0, H - 1) * W + np.clip(xi, 0, W - 1)
        flat = (p * U + uu)[valid]
        Mflat += np.bincount(flat, weights=w[valid].astype(np.float64),
                             minlength=H * W * U).astype(np.float32)
    return Mflat.reshape(H * W, U)


def _slot_axis(angs):
    th = np.array(angs, np.float64) * (np.pi / A)
    return 0 if np.mean(np.abs(np.sin(th))) <= np.mean(np.abs(np.cos(th))) else 1


def _union_geom(Ms, angs, ax):
    lo = np.full(S, U, np.int64)
    hi = np.full(S, -1, np.int64)
    for k in angs:
        Mr = Ms[k].reshape(H, W, U)
        sl = Mr if ax == 0 else Mr.transpose(1, 0, 2)
        nz = sl.any(axis=1)
        any_s = nz.any(axis=1)
        first = nz.argmax(axis=1)
        last = U - 1 - nz[:, ::-1].argmax(axis=1)
        lo = np.where(any_s, np.minimum(lo, first), lo)
        hi = np.where(any_s, np.maximum(hi, last), hi)
    wv = np.maximum(hi - lo + 1, 0).astype(np.int64)
    lo = np.where(hi < 0, 0, lo).astype(np.int64)
    return wv, lo


def _schedule(angles):
    Ms = np.stack([_build_M(np.cos(np.float64(a)), np.sin(np.float64(a)))
                   for a in angles])
    axes, widths, offs = [], [], []
    for j, (angs, blk, kind) in enumerate(SLOTS):
        ax = _slot_axis(angs)
        wv, lo = _union_geom(Ms, angs, ax)
        if kind is not None:
            base_wv, base_lo = widths[blk], offs[blk]
            assert (wv == base_wv).all(), f"slot {j}: mirror width mismatch"
            exp_lo = (np.where(base_wv > 0, U - base_lo - base_wv, lo)
                      if kind == "flip" else base_lo)
            assert (np.where(wv > 0, lo == exp_lo, True)).all(), \
                f"slot {j}: mirror offs mismatch"
            lo = exp_lo
        cover = np.zeros(U, bool)
        for s in range(S):
            if wv[s] > 0:
                cover[lo[s]:lo[s] + wv[s]] = True
        assert cover.all(), f"slot {j}: uncovered out cols"
        axes.append(ax)
        widths.append(wv)
        offs.append(lo)
    return Ms, np.array(axes), np.stack(widths), np.stack(offs)


def _group_geom(widths, offs, g):
    """Per-range G packing for a merged group.

    Returns per range r: G[k] per member, Tspan, and per slab s in range:
    (mn, mx) merged span in region coords (mn>mx if empty); plus per-member
    drain runs [(g_lo, g_hi, u_lo)] in region coords.
    """
    js = g["slots"]
    rsz = g["rsz"]
    ranges = []
    for r in range(S // rsz):
        r0, r1 = r * rsz, (r + 1) * rsz
        G = []
        cur = 0
        terr = []
        for j in js:
            ss = [s for s in range(r0, r1) if widths[j][s] > 0]
            if not ss:
                G.append(None)
                terr.append(None)
                continue
            tlo = min(int(offs[j][s]) for s in ss)
            thi = max(int(offs[j][s] + widths[j][s] - 1) for s in ss)
            G.append(cur - tlo)
            terr.append((cur, cur + thi - tlo))
            cur = cur + (thi - tlo) + 1
        Tspan = cur
        spans = []
        for s in range(r0, r1):
            mn, mx = None, None
            for k, j in enumerate(js):
                if widths[j][s] > 0:
                    a = G[k] + int(offs[j][s])
                    b = a + int(widths[j][s]) - 1
                    mn = a if mn is None else min(mn, a)
                    mx = b if mx is None else max(mx, b)
            spans.append((mn, mx) if mn is not None else None)
        # drain runs per member: covered cols within territory
        runs = []
        for k, j in enumerate(js):
            if G[k] is None:
                runs.append([])
                continue
            covered = np.zeros(Tspan, bool)
            for s in range(r0, r1):
                if widths[j][s] > 0:
                    a = G[k] + int(offs[j][s])
                    covered[a:a + int(widths[j][s])] = True
            rr = []
            i = 0
            while i < Tspan:
                if covered[i]:
                    i2 = i
                    while i2 + 1 < Tspan and covered[i2 + 1]:
                        i2 += 1
                    rr.append((i, i2, i - G[k]))
                    i = i2 + 1
                else:
                    i += 1
            runs.append(rr)
        ranges.append(dict(G=G, Tspan=Tspan, spans=spans, runs=runs, r0=r0))
    return ranges


_COMPILED = {}


def _get_compiled(angles):
    key = hash(angles.tobytes())
    if key in _COMPILED:
        return _COMPILED[key]
    from contextlib import ExitStack
    import concourse.bacc as bacc
    import concourse.tile as tile
    import concourse.mybir as mybir

    Ms, axes, widths, offs = _schedule(angles)
    for g in MGROUPS:
        ph = g["phase"]
        for j in g["slots"]:
            assert axes[j] == ph, (j, axes[j], ph)
    gg = [_group_geom(widths, offs, g) for g in MGROUPS]
    # mirror consistency: shared groups must be exact reflections (flip) or
    # identical (plain) so one stored block serves both.
    for gi, g in enumerate(MGROUPS):
        if g["store"]:
            continue
        bb = gg[g["src"]]
        mm = gg[gi]
        assert len(bb) == len(mm)
        for r in range(len(bb)):
            assert mm[r]["Tspan"] == bb[r]["Tspan"], (gi, r)
            for si in range(len(bb[r]["spans"])):
                sb_, sm_ = bb[r]["spans"][si], mm[r]["spans"][si]
                assert (sb_ is None) == (sm_ is None), (gi, r, si)
                if sb_ is None:
                    continue
                if g["kind"] == "flip":
                    Tsp = bb[r]["Tspan"]
                    assert sm_[0] == Tsp - 1 - sb_[1], (gi, r, si)
                    assert sm_[1] == Tsp - 1 - sb_[0], (gi, r, si)
                else:
                    assert sm_ == sb_, (gi, r, si)

    # out column order: phase0 merged/singles then phase1
    order = ([j for g in MGROUPS if g["phase"] == 0 for j in g["slots"]]
             + [sg["slot"] for sg in SINGLES if sg["phase"] == 0]
             + [j for g in MGROUPS if g["phase"] == 1 for j in g["slots"]]
             + [sg["slot"] for sg in SINGLES if sg["phase"] == 1])
    out_base = {j: 128 * p for p, j in enumerate(order)}
    ph0_cols = 128 * 7  # phase-0 slots: MG0(3) + MG3(3) + U0
    # merged slots get an f32 accumulator region
    acc_slots = [j for g in MGROUPS for j in g["slots"]]
    acc_base = {j: 128 * p for p, j in enumerate(acc_slots)}

    # stored column layout: m0 per slab = [MG0 span][MG2 span][U0 w];
    # m1 per slab = [U1 w][U2 w]
    def stream_layout(parts):
        # parts: list of per-slab width arrays
        base = np.zeros(S + 1, np.int64)
        cum = []
        for s in range(S):
            c = 0
            offs_ = []
            for warr in parts:
                offs_.append(c)
                c += int(warr[s])
            cum.append(offs_)
            base[s + 1] = base[s] + c
        return base, np.array(cum, np.int64), int(base[S])

    def gspan_width(gi):
        w = np.zeros(S, np.int64)
        for rr in gg[gi]:
            for si, sp in enumerate(rr["spans"]):
                if sp is not None:
                    w[rr["r0"] + si] = sp[1] - sp[0] + 1
        return w

    w_mg0 = gspan_width(0)
    w_mg2 = gspan_width(2)
    w_mg4 = gspan_width(4)
    base0, cum0, W0tot = stream_layout([w_mg0, w_mg2, widths[6]])
    base1, cum1, W1tot = stream_layout([w_mg4])

    nc = bacc.Bacc("TRN2", target_bir_lowering=False, debug=False,
                   enable_asserts=False, num_devices=NCORES)
    bf16 = mybir.dt.bfloat16
    f8 = mybir.dt.float8e3
    f32 = mybir.dt.float32

    volA_d = nc.dram_tensor("volA", [S, H * D], bf16, kind="ExternalInput").ap()
    volB_d = nc.dram_tensor("volB", [S, H * D], bf16, kind="ExternalInput").ap()
    m0_d = nc.dram_tensor("m0", [S, W0tot], f8, kind="ExternalInput").ap()
    m1_d = nc.dram_tensor("m1", [S, W1tot], f8, kind="ExternalInput").ap()
    out_d = nc.dram_tensor("out", [V, NSLOT * U], bf16, kind="ExternalOutput").ap()

    with tile.TileContext(nc) as tc:
        with ExitStack() as ctx:
            sbuf = ctx.enter_context(tc.tile_pool(name="sbuf", bufs=1))
            psum = ctx.enter_context(tc.tile_pool(name="psum", bufs=1, space="PSUM"))

            volA_sb = sbuf.tile([S, H * D], bf16)
            volB_sb = sbuf.tile([S, H * D], bf16)
            m0_sb = sbuf.tile([S, W0tot], f8)
            m1_sb = sbuf.tile([S, W1tot], f8)
            acc_sb = sbuf.tile([V, len(acc_slots) * U], f32)
            out_sb = sbuf.tile([V, NSLOT * U], bf16)
            ps = [psum.tile([V, 512], f32, name=f"ps{b}") for b in range(8)]

            # psum bank plan:
            #  MG0 (ph0): ps0/ps1   MG3 (ph0): ps2/ps3   U0: ps4[0:128]
            #  MG1 (ph1): ps5/ps6   MG2 (ph1): ps0/ps1   MG4 (ph1): ps2/ps3
            #  (cross-phase reuse is safe: the new epoch's matmuls are
            #  ordered behind the old epoch's full-bank scratch copy)
            mg_banks = {0: (0, 1, 2), 3: (5, 6, 7), 1: (5, 6, 7),
                        2: (0, 1, 2), 4: (3, 4)}
            single_bank = {6: (4, 0)}
            bank_w = {b: 136 for b in range(8)}
            for gi2, bks in mg_banks.items():
                mt = max(rr2["Tspan"] for rr2 in gg[gi2])
                for b in bks:
                    bank_w[b] = min(512, max(bank_w[b], mt + 8))

            # zero the merged-slot accumulator (DVE+ACT halves)
            hn = len(acc_slots) * U // 2
            nc.vector.memset(acc_sb[:, 0:hn], 0.0)
            nc.scalar.memzero(acc_sb[:, hn:len(acc_slots) * U])
            # zero all psum banks once: every matmul runs start=False and
            # accumulates onto zeroed psum; epoch boundaries re-zero via the
            # ACT anchor below, so psum first-touch state never matters.
            for b in range(8):
                if b % 2 == 0:
                    nc.vector.memset(ps[b], 0.0)
                else:
                    nc.scalar.memzero(ps[b])
            WARMUP = 8
            for _ in range(WARMUP):
                nc.tensor.matmul(ps[7][:, 0:128], lhsT=acc_sb[:, 0:128],
                                 rhs=acc_sb[:, 128:256], start=True, stop=True)

            # ---- stream DMAs: few big dma_starts, need-ordered, 2 rings
            def vitem(sb_t, d_t, s0, s1):
                return (sb_t[:, s0 * H:s1 * H], d_t[:, s0 * H:s1 * H])

            def mitem(sb_t, d_t, base, s0, s1):
                c0, c1 = int(base[s0]), int(base[s1])
                return (sb_t[:, c0:c1], d_t[:, c0:c1]) if c1 > c0 else None

            ring0 = [vitem(volA_sb, volA_d, 0, 2),
                     mitem(m0_sb, m0_d, base0, 2, 18),
                     vitem(volA_sb, volA_d, 18, 44),
                     mitem(m0_sb, m0_d, base0, 44, 74),
                     vitem(volA_sb, volA_d, 74, 102),
                     mitem(m0_sb, m0_d, base0, 102, 128),
                     vitem(volB_sb, volB_d, 0, 24),
                     mitem(m1_sb, m1_d, base1, 24, 64),
                     vitem(volB_sb, volB_d, 64, 128)]
            ring1 = [mitem(m0_sb, m0_d, base0, 0, 2),
                     vitem(volA_sb, volA_d, 2, 18),
                     mitem(m0_sb, m0_d, base0, 18, 44),
                     vitem(volA_sb, volA_d, 44, 74),
                     mitem(m0_sb, m0_d, base0, 74, 102),
                     vitem(volA_sb, volA_d, 102, 128),
                     mitem(m1_sb, m1_d, base1, 0, 24),
                     vitem(volB_sb, volB_d, 24, 64),
                     mitem(m1_sb, m1_d, base1, 64, 128)]
            for eng, items in ((nc.sync, ring0), (nc.scalar, ring1)):
                for it in items:
                    if it is not None:
                        eng.dma_start(it[0], it[1])

            # ---- start/stop: every (group, range) opens with start=True
            # (clears its ping/pong bank; safe — the prior epoch was drained,
            # and Tile serializes the PE write behind the drain reads) and
            # closes with stop=True on its last matmul.  Singles likewise per
            # slot (each owns its bank region during its phase).
            for phase in (0, 1):
                vol_sb = volA_sb if phase == 0 else volB_sb
                mgs = [gi for gi in range(len(MGROUPS))
                       if MGROUPS[gi]["phase"] == phase]
                sgs = [sg for sg in SINGLES if sg["phase"] == phase]

                def drain_range(gi, rr, bank):
                    grp = MGROUPS[gi]
                    for k, j in enumerate(grp["slots"]):
                        for (g_lo, g_hi, u_lo) in rr["runs"][k]:
                            n = g_hi - g_lo + 1
                            ab = acc_base[j] + u_lo
                            nc.vector.tensor_add(
                                acc_sb[:, ab:ab + n],
                                acc_sb[:, ab:ab + n],
                                ps[bank][:, g_lo:g_hi + 1])
                    # DVE re-zeroes the bank's active region right after its
                    # own drain reads (same engine -> no cross-engine sem):
                    # a full-region write Tile orders after this epoch's
                    # matmuls AND the drain reads, and before the next
                    # epoch's matmuls (all matmuls run start=False onto
                    # zeroed psum).  With the 3-deep bank rotation this
                    # anchor sits ~2 epochs off the tensor critical path.
                    nc.vector.memset(ps[bank][:, 0:bank_w[bank]], 0.0)

                for s in range(S):
                    lhsT = vol_sb[:, s * D:(s + 1) * D]
                    for gi in mgs:
                        grp = MGROUPS[gi]
                        rsz = grp["rsz"]
                        r, si = s // rsz, s % rsz
                        rr = gg[gi][r]
                        sp = rr["spans"][si]
                        if sp is None:
                            continue
                        mn, mx = sp
                        L = mx - mn + 1
                        bank = mg_banks[gi][r % len(mg_banks[gi])]
                        if grp["src"] == 4:
                            c0 = int(base1[s] + cum1[s][0])
                            src_sb = m1_sb
                        else:
                            part = 0 if grp["src"] == 0 else 1
                            c0 = int(base0[s] + cum0[s][part])
                            src_sb = m0_sb
                        rhs = src_sb[:, c0:c0 + L]
                        if grp["kind"] == "flip":
                            rhs = rhs[:, ::-1]
                        is_last = not any(
                            rr["spans"][si2] is not None
                            for si2 in range(si + 1, rsz))
                        nc.tensor.matmul(
                            ps[bank][:, mn:mx + 1], lhsT=lhsT, rhs=rhs,
                            start=False, stop=is_last,
                            skip_group_check=True)
                    for sg in sgs:
                        j = sg["slot"]
                        w = int(widths[j][s])
                        if w == 0:
                            continue
                        bank, cb = single_bank[j]
                        col = cb + int(offs[j][s])
                        c0 = int(base0[s] + cum0[s][2])
                        is_last = not any(
                            widths[j][s2] > 0 for s2 in range(s + 1, S))
                        nc.tensor.matmul(
                            ps[bank][:, col:col + w], lhsT=lhsT,
                            rhs=m0_sb[:, c0:c0 + w], start=False,
                            stop=is_last, skip_group_check=True)
                    # fire drains for every group whose range ends at slab s
                    for gi in mgs:
                        rsz = MGROUPS[gi]["rsz"]
                        if (s + 1) % rsz == 0:
                            r = s // rsz
                            drain_range(gi, gg[gi][r],
                                        mg_banks[gi][r % len(mg_banks[gi])])
                # ---- end of phase: singles drain + merged convert + flush
                for sg in sgs:
                    j = sg["slot"]
                    bank, cb = single_bank[j]
                    nc.scalar.mul(out_sb[:, out_base[j]:out_base[j] + U],
                                  ps[bank][:, cb:cb + U], 1.0)
                    nc.scalar.memzero(ps[bank][:, 0:bank_w[bank]])
                for qe, gi in enumerate(mgs):
                    js = MGROUPS[gi]["slots"]
                    ob = out_base[js[0]]
                    ab = acc_base[js[0]]
                    n = len(js) * U
                    assert all(out_base[j2] == ob + 128 * i2
                               for i2, j2 in enumerate(js))
                    assert all(acc_base[j2] == ab + 128 * i2
                               for i2, j2 in enumerate(js))
                    if qe % 2 == 0:
                        nc.vector.tensor_scalar_mul(
                            out_sb[:, ob:ob + n], acc_sb[:, ab:ab + n], 1.0)
                    else:
                        nc.scalar.mul(out_sb[:, ob:ob + n],
                                      acc_sb[:, ab:ab + n], 1.0)
                lo_, hi_ = (0, ph0_cols) if phase == 0 else (ph0_cols, NSLOT * U)
                mid = (lo_ + hi_) // 2
                nc.sync.dma_start(out_d[:, lo_:mid], out_sb[:, lo_:mid])
                nc.scalar.dma_start(out_d[:, mid:hi_], out_sb[:, mid:hi_])

    nc.compile()
    meta = dict(nc=nc, Ms=Ms, axes=axes, widths=widths, offs=offs,
                gg=gg, order=order, out_base=out_base,
                lay=(base0, cum0, W0tot, base1, cum1, W1tot),
                w_mg=(w_mg0, w_mg2))
    _COMPILED[key] = meta
    return meta


def _pack_core(meta, core):
    """fp8 stored blocks (pre-scaled by DT) for one core."""
    f8 = ml_dtypes.float8_e3m4
    Ms, axes, widths, offs = (meta["Ms"], meta["axes"], meta["widths"],
                              meta["offs"])
    base0, cum0, W0tot, base1, cum1, W1tot = meta["lay"]
    gg = meta["gg"]
    m0 = np.zeros((S, W0tot), f8)
    m1 = np.zeros((S, W1tot), f8)

    def band(j, s):
        k = SLOTS[j][0][core]
        ax = axes[j]
        Mr = Ms[k].reshape(H, W, U)
        sl = Mr if ax == 0 else Mr.transpose(1, 0, 2)
        o = int(offs[j][s])
        w = int(widths[j][s])
        return (DT * sl[s][:, o:o + w]).astype(f8)

    # merged stored groups: MG0 (m0 part 0), MG2 (m0 part 1), MG4 (m1)
    for gi, (m, base, cum, part) in ((0, (m0, base0, cum0, 0)),
                                     (2, (m0, base0, cum0, 1)),
                                     (4, (m1, base1, cum1, 0))):
        grp = MGROUPS[gi]
        for rr in gg[gi]:
            for si, sp in enumerate(rr["spans"]):
                s = rr["r0"] + si
                if sp is None:
                    continue
                mn = sp[0]
                c0 = int(base[s] + cum[s][part])
                for k, j in enumerate(grp["slots"]):
                    if widths[j][s] > 0:
                        a = rr["G"][k] + int(offs[j][s])
                        m[:, c0 + a - mn:c0 + a - mn + int(widths[j][s])] = \
                            band(j, s)
    # single U0
    for s in range(S):
        w = int(widths[6][s])
        if w > 0:
            c0 = int(base0[s] + cum0[s][2])
            m0[:, c0:c0 + w] = band(6, s)
    return m0, m1


def kernel(vol, angles):
    from concourse.bass_utils import run_bass_kernel_spmd

    vol = np.asarray(vol, dtype=np.float32)
    angles = np.asarray(angles, dtype=np.float32)
    meta = _get_compiled(angles)
    nc = meta["nc"]

    volA = vol[0, 0].reshape(S, H * D).astype(ml_dtypes.bfloat16)
    volB = np.ascontiguousarray(vol[0, 0].transpose(1, 0, 2)).reshape(
        S, H * D).astype(ml_dtypes.bfloat16)
    in_maps = []
    for c in range(NCORES):
        m0, m1 = _pack_core(meta, c)
        in_maps.append({"volA": volA, "volB": volB, "m0": m0, "m1": m1})

    res = run_bass_kernel_spmd(nc, in_maps, core_ids=list(range(NCORES)))
    global _LAST_RES
    _LAST_RES = res
    full = np.empty((1, 1, U, A, V), np.float32)
    for c, r in enumerate(res.results):
        rc = r["out"].astype(np.float32)
        for p, j in enumerate(meta["order"]):
            k = SLOTS[j][0][c]
            full[0, 0, :, k, :] = rc[:, p * U:(p + 1) * U].T
    return full
